# revision 2
# baseline (speedup 1.0000x reference)
"""GCN layer (gnn_message_passing) Trainium2 Bass kernel.

Problem: out[b,n,:] = relu( sum_r (mean_k padded[b, idx[b,r,n,k]]) @ W_r
                            + feat[b,n] @ W_self + bias )
  B=4, N=4096, D=O=128, R=4, K=16.

Strategy: shard (batch x N-half) across 8 cores -> no collectives.
Per core (b, h):
  - DRAM table tbl[4097, 128] bf16 = [zeros; node_features[b]] (host-cast).
  - SWDGE dma_gather (transpose=True) pulls neighbor rows as columns
    [d, j] in bf16; relation r's stream is idx[b,r,n,k] in natural order
    (k innermost), the "self" stream is n+1.
  - DVE tensor_reduce sums k (innermost 16) -> aggT_r [d, n] f32.
  - PE: out_psum[n, o] = sum_r aggT_r_slice.T @ (W_r/K) + selfT.T @ W_self
        + ones.T @ bias  (f32 matmuls, accumulated in PSUM).
  - ACT applies ReLU, HWDGE stores [n, o] f32 rows.
"""

import numpy as np
import ml_dtypes

import concourse.bacc as bacc
import concourse.mybir as mybir
from concourse.tile import TileContext
from concourse.bass_utils import run_bass_kernel_spmd

B, N, D = 4, 4096, 128
R, K, O = 4, 16, 128
NCORES = 8
NH = N // 2            # nodes per core
CHUNK = 512            # nodes per chunk
NCH = NH // CHUNK      # chunks per core
RJ = CHUNK * K         # idxs per relation-call (8192)
SEG = R * RJ // 16 + CHUNK // 16   # idx cols per chunk: 4*512 + 32 = 2080
G_BUFS = 6

_cache = {}


def _build():
    nc = bacc.Bacc("TRN2")
    tbl = nc.dram_tensor("tbl", [N + 1, D], mybir.dt.bfloat16, kind="ExternalInput")
    idxs = nc.dram_tensor("idxs", [128, NCH * SEG], mybir.dt.int16, kind="ExternalInput")
    w = nc.dram_tensor("w", [128, R + 2, O], mybir.dt.float32, kind="ExternalInput")
    out = nc.dram_tensor("out", [NH, O], mybir.dt.float32, kind="ExternalOutput")

    with TileContext(nc) as tc:
        with (
            tc.tile_pool(name="const", bufs=1) as cpool,
            tc.tile_pool(name="idx", bufs=2) as ipool,
            tc.tile_pool(name="g", bufs=G_BUFS) as gpool,
            tc.tile_pool(name="gs", bufs=2) as gspool,
            tc.tile_pool(name="agg", bufs=6) as apool,
            tc.tile_pool(name="aggs", bufs=2) as aspool,
            tc.tile_pool(name="osb", bufs=2) as opool,
            tc.tile_pool(name="ps", bufs=8, space="PSUM") as pspool,
        ):
            w_sb = cpool.tile([128, R + 2, O], mybir.dt.float32)
            nc.sync.dma_start(w_sb[:], w[:])
            ones = cpool.tile([1, 128], mybir.dt.float32)
            nc.vector.memset(ones[:], 1.0)

            for ch in range(NCH):
                idx_sb = ipool.tile([128, SEG], mybir.dt.int16)
                nc.sync.dma_start(idx_sb[:], idxs[:, ch * SEG:(ch + 1) * SEG])

                aggs = []
                for r in range(R):
                    g = gpool.tile([128, 1, RJ], mybir.dt.bfloat16, tag="g")
                    nc.gpsimd.dma_gather(
                        g[:], tbl[:],
                        idx_sb[:, r * (RJ // 16):(r + 1) * (RJ // 16)],
                        RJ, RJ, D, transpose=True, single_packet=False,
                    )
                    aggf = apool.tile([128, CHUNK], mybir.dt.float32, tag="aggf")
                    nc.vector.tensor_reduce(
                        aggf[:],
                        g[:, 0, :].rearrange("p (n k) -> p n k", k=K),
                        mybir.AxisListType.X,
                        mybir.AluOpType.add,
                    )
                    aggs.append(aggf)

                g_s = gspool.tile([128, 1, CHUNK], mybir.dt.bfloat16, tag="gs")
                nc.gpsimd.dma_gather(
                    g_s[:], tbl[:],
                    idx_sb[:, R * (RJ // 16):],
                    CHUNK, CHUNK, D, transpose=True, single_packet=False,
                )
                agg_s = aspool.tile([128, CHUNK], mybir.dt.float32, tag="aggsf")
                nc.vector.tensor_copy(agg_s[:], g_s[:, 0, :])

                out_sb = opool.tile([128, CHUNK // 128, O], mybir.dt.float32)
                for t in range(CHUNK // 128):
                    ps = pspool.tile([128, O], mybir.dt.float32)
                    sl = slice(t * 128, (t + 1) * 128)
                    for r in range(R):
                        nc.tensor.matmul(
                            ps[:], aggs[r][:, sl], w_sb[:, r, :],
                            start=(r == 0), stop=False,
                        )
                    nc.tensor.matmul(
                        ps[:], agg_s[:, sl], w_sb[:, R, :],
                        start=False, stop=False,
                    )
                    nc.tensor.matmul(
                        ps[:], ones[:1, :], w_sb[0:1, R + 1, :],
                        start=False, stop=True,
                    )
                    nc.scalar.activation(
                        out_sb[:, t, :], ps[:], mybir.ActivationFunctionType.Relu
                    )
                nc.sync.dma_start(
                    out[ch * CHUNK:(ch + 1) * CHUNK, :].rearrange(
                        "(t p) o -> p t o", p=128
                    ),
                    out_sb[:],
                )

    nc.compile()
    return nc


def _prep_inputs(node_features, neighbor_indices, relation_kernels, self_kernel, bias):
    """Host-side shard/layout prep. Returns per-core input maps."""
    nf = np.asarray(node_features)
    idx = np.asarray(neighbor_indices)
    in_maps = []
    tbls = []
    for b in range(B):
        t = np.zeros((N + 1, D), dtype=ml_dtypes.bfloat16)
        t[1:] = nf[b].astype(ml_dtypes.bfloat16)
        tbls.append(t)

    w = np.zeros((128, R + 2, O), dtype=np.float32)
    for r in range(R):
        w[:, r, :] = np.asarray(relation_kernels)[r] / K
    w[:, R, :] = np.asarray(self_kernel)
    w[0, R + 1, :] = np.asarray(bias)

    for c in range(NCORES):
        b, h = divmod(c, 2)
        base = h * NH
        cols = np.empty((16, NCH * SEG), dtype=np.int16)
        for ch in range(NCH):
            seg = np.empty((16, SEG), dtype=np.int16)
            for r in range(R):
                stream = idx[b, r, base + ch * CHUNK: base + (ch + 1) * CHUNK, :]
                stream = stream.reshape(-1).astype(np.int16)
                seg[:, r * (RJ // 16):(r + 1) * (RJ // 16)] = stream.reshape(-1, 16).T
            selfs = np.arange(base + ch * CHUNK + 1, base + (ch + 1) * CHUNK + 1,
                              dtype=np.int16)
            seg[:, R * (RJ // 16):] = selfs.reshape(-1, 16).T
            cols[:, ch * SEG:(ch + 1) * SEG] = seg
        in_maps.append({
            "tbl": tbls[b],
            "idxs": np.tile(cols, (8, 1)),
            "w": w,
        })
    return in_maps


def _run(in_maps, **kw):
    if "nc" not in _cache:
        _cache["nc"] = _build()
    return run_bass_kernel_spmd(_cache["nc"], in_maps, core_ids=list(range(NCORES)), **kw)


def _assemble(results):
    out = np.empty((B, N, O), dtype=np.float32)
    for c in range(NCORES):
        b, h = divmod(c, 2)
        out[b, h * NH:(h + 1) * NH, :] = results[c]["out"]
    return out


def kernel(node_features, neighbor_indices, relation_kernels, self_kernel, bias):
    in_maps = _prep_inputs(node_features, neighbor_indices, relation_kernels,
                           self_kernel, bias)
    res = _run(in_maps)
    return _assemble(res.results)



# revision 7
# speedup vs baseline: 6.0073x; 6.0073x over previous
"""GCN layer (gnn_message_passing) Trainium2 Bass kernel.

Problem: out[b,n,:] = relu( sum_r (mean_k padded[b, idx[b,r,n,k]]) @ W_r
                            + feat[b,n] @ W_self + bias )
  B=4, N=4096, D=O=128, R=4, K=16.

Strategy: shard (batch x N-half) across 8 cores -> no collectives.

Instead of per-edge DMA gathers (descriptor generation on the Pool/Q7
engine costs ~8ns per index -> ~1.1ms/core floor), the neighbor
aggregation is reformulated as a dense matmul with a host-packed
multi-hot count matrix:

  out.T[o, n] = sum_i T_all[i, o] * M[i, n] + W_self.T @ X.T + bias

where T_all = [padded @ W_r / K for r in 0..3] stacked ([4*4224, 128],
built on-device on the PE from the transposed feature table), and
M[r*4224 + row, n] = #{k : idx[b,r,n,k] == row}  (counts 0..16, exact
in bf16). M streams from HBM (~69MB/core) at line rate and feeds the
idle tensor engine; the Pool engine is not used at all.

Per core (b, h): 4 chunks of 512 nodes; per chunk the 132 i-tiles of
M stream in 4 double-buffered groups of 33, accumulated into one
[o=128, n=512] PSUM bank, + self matmul; ReLU+bias on ACT; store
transposed, host re-transposes.
"""

import numpy as np
import ml_dtypes

import concourse.bacc as bacc
import concourse.mybir as mybir
from concourse.tile import TileContext
from concourse.bass_utils import run_bass_kernel_spmd

B, N, D = 4, 4096, 128
R, K, O = 4, 16, 128
NCORES = 8
NH = N // 2            # nodes per core
CH = 512               # nodes per chunk
NCH = NH // CH         # chunks per core (4)
RT = 4224              # padded table rows per relation (33 * 128)
TILES = R * (RT // 128)   # 132 i-tiles
G = 33                 # tiles per M load group
NG = TILES // G        # 4 groups per chunk

_cache = {}


def _build():
    nc = bacc.Bacc("TRN2")
    tblT = nc.dram_tensor("tblT", [128, RT], mybir.dt.bfloat16, kind="ExternalInput")
    selfT = nc.dram_tensor("selfT", [128, NH], mybir.dt.bfloat16, kind="ExternalInput")
    w = nc.dram_tensor("w", [128, R + 1, O], mybir.dt.bfloat16, kind="ExternalInput")
    bias = nc.dram_tensor("bias", [128, 1], mybir.dt.float32, kind="ExternalInput")
    m_in = nc.dram_tensor("m", [128, NCH, TILES, CH], mybir.dt.bfloat16,
                          kind="ExternalInput")
    out = nc.dram_tensor("out", [128, NCH, CH], mybir.dt.float32,
                         kind="ExternalOutput")

    with TileContext(nc) as tc:
        with (
            tc.tile_pool(name="const", bufs=1) as cpool,
            tc.tile_pool(name="m", bufs=2) as mpool,
            tc.tile_pool(name="o", bufs=2) as opool,
            tc.tile_pool(name="pst", bufs=4, space="PSUM") as ptpool,
            tc.tile_pool(name="ps", bufs=2, space="PSUM") as pspool,
        ):
            tblT_sb = cpool.tile([128, RT], mybir.dt.bfloat16)
            nc.sync.dma_start(tblT_sb[:], tblT[:])
            selfT_sb = cpool.tile([128, NH], mybir.dt.bfloat16)
            nc.sync.dma_start(selfT_sb[:], selfT[:])
            w_sb = cpool.tile([128, R + 1, O], mybir.dt.bfloat16)
            nc.sync.dma_start(w_sb[:], w[:])
            bias_sb = cpool.tile([128, 1], mybir.dt.float32)
            nc.sync.dma_start(bias_sb[:], bias[:])

            # T_all[i, t, o]: per-relation transformed tables, bf16
            t_all = cpool.tile([128, TILES, O], mybir.dt.bfloat16)
            for r in range(R):
                for t in range(RT // 128):
                    pt = ptpool.tile([128, O], mybir.dt.float32)
                    nc.tensor.matmul(
                        pt[:], tblT_sb[:, t * 128:(t + 1) * 128], w_sb[:, r, :],
                        start=True, stop=True,
                    )
                    nc.vector.tensor_copy(t_all[:, r * (RT // 128) + t, :], pt[:])

            for ch in range(NCH):
                ps = pspool.tile([128, CH], mybir.dt.float32)
                # self contribution first (no M dependency)
                nc.tensor.matmul(
                    ps[:], w_sb[:, R, :],
                    selfT_sb[:, ch * CH:(ch + 1) * CH],
                    start=True, stop=False,
                )
                for g in range(NG):
                    m_sb = mpool.tile([128, G, CH], mybir.dt.bfloat16)
                    nc.sync.dma_start(m_sb[:], m_in[:, ch, g * G:(g + 1) * G, :])
                    for tl in range(G):
                        t = g * G + tl
                        nc.tensor.matmul(
                            ps[:], t_all[:, t, :], m_sb[:, tl, :],
                            start=False, stop=(t == TILES - 1),
                        )
                out_sb = opool.tile([128, CH], mybir.dt.float32)
                nc.scalar.activation(
                    out_sb[:], ps[:], mybir.ActivationFunctionType.Relu,
                    bias=bias_sb[:],
                )
                nc.sync.dma_start(out[:, ch, :], out_sb[:])

    nc.compile()
    return nc


def _prep_inputs(node_features, neighbor_indices, relation_kernels, self_kernel, bias):
    """Host-side shard/layout prep. Returns per-core input maps."""
    nf = np.asarray(node_features)
    idx = np.asarray(neighbor_indices)

    w = np.zeros((128, R + 1, O), dtype=ml_dtypes.bfloat16)
    for r in range(R):
        w[:, r, :] = (np.asarray(relation_kernels)[r] / K).astype(ml_dtypes.bfloat16)
    w[:, R, :] = np.asarray(self_kernel).astype(ml_dtypes.bfloat16)
    bias_col = np.asarray(bias).astype(np.float32).reshape(128, 1)

    tblTs = []
    for b in range(B):
        t = np.zeros((128, RT), dtype=ml_dtypes.bfloat16)
        t[:, 1:N + 1] = nf[b].T.astype(ml_dtypes.bfloat16)
        tblTs.append(t)

    in_maps = []
    cols = np.repeat(np.arange(NH, dtype=np.int64), K)
    for c in range(NCORES):
        b, h = divmod(c, 2)
        base = h * NH
        cnt = np.zeros((R * RT, NH), dtype=np.uint8)
        for r in range(R):
            lin = (r * RT + idx[b, r, base:base + NH, :].astype(np.int64)).ravel()
            np.add.at(cnt, (lin, cols), 1)
        m = cnt.reshape(TILES, 128, NCH, CH).transpose(1, 2, 0, 3)
        in_maps.append({
            "tblT": tblTs[b],
            "selfT": np.ascontiguousarray(tblTs[b][:, 1 + base:1 + base + NH]),
            "w": w,
            "bias": bias_col,
            "m": np.ascontiguousarray(m).astype(ml_dtypes.bfloat16),
        })
    return in_maps


def _run(in_maps, **kw):
    if "nc" not in _cache:
        _cache["nc"] = _build()
    return run_bass_kernel_spmd(_cache["nc"], in_maps, core_ids=list(range(NCORES)), **kw)


def _assemble(results):
    out = np.empty((B, N, O), dtype=np.float32)
    for c in range(NCORES):
        b, h = divmod(c, 2)
        o = results[c]["out"]  # [128, NCH, CH] = [o, ch, n]
        out[b, h * NH:(h + 1) * NH, :] = o.transpose(1, 2, 0).reshape(NH, O)
    return out


def kernel(node_features, neighbor_indices, relation_kernels, self_kernel, bias):
    in_maps = _prep_inputs(node_features, neighbor_indices, relation_kernels,
                           self_kernel, bias)
    res = _run(in_maps)
    return _assemble(res.results)


# revision 11
# speedup vs baseline: 9.3039x; 1.5488x over previous
"""GCN layer (gnn_message_passing) Trainium2 Bass kernel.

Problem: out[b,n,:] = relu( sum_r (mean_k padded[b, idx[b,r,n,k]]) @ W_r
                            + feat[b,n] @ W_self + bias )
  B=4, N=4096, D=O=128, R=4, K=16.

Strategy: shard (batch x N-half) across 8 cores -> no collectives.

Per-edge DMA gathers cost ~8ns/index of Q7 descriptor generation on the
Pool engine (~1.1ms/core floor), so the neighbor aggregation is instead
a dense matmul with a host-packed multi-hot count matrix:

  out.T[o, n] = sum_i T_all[i, o] * M[i, n] + W_self.T @ X.T + bias

T_all = [padded @ W_r / K, r=0..3] stacked ([4*4224, 128] bf16, built
on-device on the PE). M[r*4224 + row, n] = #{k : idx[b,r,n,k] == row}
in fp8e4 (counts 0..16 are exact; halves HBM traffic vs bf16). M
streams at line rate into the tensor engine; Pool is unused.

Loop order is i-tile-major with all 4 node-chunk accumulators live in
separate PSUM banks, so each T_all tile is loaded once per 4 matmuls.
ReLU+bias fuse on ACT (bias per-partition, exact f32); output leaves
transposed [o, n] and the host re-transposes.
"""

import numpy as np
import ml_dtypes

import concourse.bacc as bacc
import concourse.mybir as mybir
from concourse.tile import TileContext
from concourse.bass_utils import run_bass_kernel_spmd

B, N, D = 4, 4096, 128
R, K, O = 4, 16, 128
NCORES = 8
NH = N // 2            # nodes per core
CH = 512               # nodes per chunk (one PSUM bank)
NCH = NH // CH         # chunks per core (4)
RT = 4224              # padded table rows per relation (33 * 128)
TILES = R * (RT // 128)   # 132 i-tiles
GT = 12                # i-tiles per M load group
NG = TILES // GT       # 11 groups

M_DT = mybir.dt.float8e4
M_NP = ml_dtypes.float8_e4m3

_cache = {}


def _build():
    nc = bacc.Bacc("TRN2")
    tblT = nc.dram_tensor("tblT", [128, RT], mybir.dt.bfloat16, kind="ExternalInput")
    selfT = nc.dram_tensor("selfT", [128, NH], mybir.dt.bfloat16, kind="ExternalInput")
    w = nc.dram_tensor("w", [128, R + 1, O], mybir.dt.bfloat16, kind="ExternalInput")
    bias = nc.dram_tensor("bias", [128, 1], mybir.dt.float32, kind="ExternalInput")
    m_in = nc.dram_tensor("m", [128, TILES, NH], M_DT, kind="ExternalInput")
    out = nc.dram_tensor("out", [128, NCH, CH], mybir.dt.float32,
                         kind="ExternalOutput")

    with TileContext(nc) as tc:
        with (
            tc.tile_pool(name="const", bufs=1) as cpool,
            tc.tile_pool(name="m", bufs=2) as mpool,
            tc.tile_pool(name="o", bufs=2) as opool,
            tc.tile_pool(name="pst", bufs=4, space="PSUM") as ptpool,
            tc.tile_pool(name="ps", bufs=1, space="PSUM") as pspool,
        ):
            tblT_sb = cpool.tile([128, RT], mybir.dt.bfloat16)
            nc.sync.dma_start(tblT_sb[:], tblT[:])
            selfT_sb = cpool.tile([128, NH], mybir.dt.bfloat16)
            nc.sync.dma_start(selfT_sb[:], selfT[:])
            w_sb = cpool.tile([128, R + 1, O], mybir.dt.bfloat16)
            nc.sync.dma_start(w_sb[:], w[:])
            bias_sb = cpool.tile([128, 1], mybir.dt.float32)
            nc.sync.dma_start(bias_sb[:], bias[:])

            # T_all[i, t, o]: per-relation transformed tables, bf16
            t_all = cpool.tile([128, TILES, O], mybir.dt.bfloat16)
            for r in range(R):
                for t in range(RT // 128):
                    pt = ptpool.tile([128, O], mybir.dt.float32)
                    nc.tensor.matmul(
                        pt[:], tblT_sb[:, t * 128:(t + 1) * 128], w_sb[:, r, :],
                        start=True, stop=True,
                    )
                    nc.vector.tensor_copy(t_all[:, r * (RT // 128) + t, :], pt[:])

            # 4 live accumulators, one per node chunk
            ps = [pspool.tile([128, CH], mybir.dt.float32, name=f"ps{i}",
                              tag=f"ps{i}")
                  for i in range(NCH)]
            for ch in range(NCH):
                nc.tensor.matmul(
                    ps[ch][:], w_sb[:, R, :],
                    selfT_sb[:, ch * CH:(ch + 1) * CH],
                    start=True, stop=False, skip_group_check=True,
                )
            for g in range(NG):
                m_sb = mpool.tile([128, GT, NH], M_DT)
                nc.sync.dma_start(m_sb[:], m_in[:, g * GT:(g + 1) * GT, :])
                for tl in range(GT):
                    t = g * GT + tl
                    for ch in range(NCH):
                        nc.tensor.matmul(
                            ps[ch][:], t_all[:, t, :],
                            m_sb[:, tl, ch * CH:(ch + 1) * CH],
                            start=False, stop=(t == TILES - 1),
                            skip_group_check=True,
                        )
            for ch in range(NCH):
                out_sb = opool.tile([128, CH], mybir.dt.float32)
                nc.scalar.activation(
                    out_sb[:], ps[ch][:], mybir.ActivationFunctionType.Relu,
                    bias=bias_sb[:],
                )
                nc.sync.dma_start(out[:, ch, :], out_sb[:])

    nc.compile()
    return nc


def _prep_inputs(node_features, neighbor_indices, relation_kernels, self_kernel, bias):
    """Host-side shard/layout prep. Returns per-core input maps."""
    nf = np.asarray(node_features)
    idx = np.asarray(neighbor_indices)

    w = np.zeros((128, R + 1, O), dtype=ml_dtypes.bfloat16)
    for r in range(R):
        w[:, r, :] = (np.asarray(relation_kernels)[r] / K).astype(ml_dtypes.bfloat16)
    w[:, R, :] = np.asarray(self_kernel).astype(ml_dtypes.bfloat16)
    bias_col = np.asarray(bias).astype(np.float32).reshape(128, 1)

    tblTs = []
    for b in range(B):
        t = np.zeros((128, RT), dtype=ml_dtypes.bfloat16)
        t[:, 1:N + 1] = nf[b].T.astype(ml_dtypes.bfloat16)
        tblTs.append(t)

    in_maps = []
    cols = np.repeat(np.arange(NH, dtype=np.int64), K)
    for c in range(NCORES):
        b, h = divmod(c, 2)
        base = h * NH
        cnt = np.zeros((R * RT, NH), dtype=np.uint8)
        for r in range(R):
            lin = (r * RT + idx[b, r, base:base + NH, :].astype(np.int64)).ravel()
            np.add.at(cnt, (lin, cols), 1)
        m = cnt.reshape(TILES, 128, NH).transpose(1, 0, 2)
        in_maps.append({
            "tblT": tblTs[b],
            "selfT": np.ascontiguousarray(tblTs[b][:, 1 + base:1 + base + NH]),
            "w": w,
            "bias": bias_col,
            "m": np.ascontiguousarray(m).astype(M_NP),
        })
    return in_maps


def _run(in_maps, **kw):
    if "nc" not in _cache:
        _cache["nc"] = _build()
    return run_bass_kernel_spmd(_cache["nc"], in_maps, core_ids=list(range(NCORES)), **kw)


def _assemble(results):
    out = np.empty((B, N, O), dtype=np.float32)
    for c in range(NCORES):
        b, h = divmod(c, 2)
        o = results[c]["out"]  # [128, NCH, CH] = [o, ch, n]
        out[b, h * NH:(h + 1) * NH, :] = o.transpose(1, 2, 0).reshape(NH, O)
    return out


def kernel(node_features, neighbor_indices, relation_kernels, self_kernel, bias):
    in_maps = _prep_inputs(node_features, neighbor_indices, relation_kernels,
                           self_kernel, bias)
    res = _run(in_maps)
    return _assemble(res.results)
